# revision 1
# baseline (speedup 1.0000x reference)
"""Trainium2 Bass kernel for a 12-layer dense transformer encoder.

Sharding: data-parallel over batch B=8 across the 8 NeuronCores (one batch
element per core, weights replicated, no collectives).

Per-core kernel: full transformer forward for one [T=1024, D=1024] sequence.
All matmuls run in fp32r (full-rate reduced-precision fp32 on the PE).
LayerNorm gamma and the 1/sqrt(D) attention scale are folded into the weight
matrices host-side; LayerNorm beta and the MLP biases become per-output bias
vectors applied on-device (per-partition biases via tensor_scalar/activation,
free-dim biases via broadcast adds or rank-1 ones-matmuls).
"""

import numpy as np

import concourse.bass as bass
import concourse.bacc as bacc
import concourse.mybir as mybir
import concourse.tile as tile
from concourse.bass_utils import run_bass_kernel_spmd
F32 = mybir.dt.float32
F32R = mybir.dt.float32r
AX = mybir.AxisListType.X
OP = mybir.AluOpType
AF = mybir.ActivationFunctionType

B = 8
T = 1024
NI = 512
VOCAB = 10
D = 1024
HM = 4096
EPS = 1e-5
P = 128
NT = T // P    # 8 token tiles
ND = D // P    # 8 embed tiles
NH = HM // P   # 32 hidden ptiles
HC = 8         # hidden chunks of 512
N2 = D // 512  # 2 free-dim chunks of 512

_CACHE = {}


def _ln_xhat(nc, pools, h_slice, xh):
    """LayerNorm stats + normalize: xh = (h - mean) * rsqrt(var + eps), fp32r."""
    st, eps_sb = pools
    stats = st.tile([P, 2, 6], F32, tag="stats")
    nc.vector.bn_stats(stats[:, 0, :], h_slice[:, 0:512])
    nc.vector.bn_stats(stats[:, 1, :], h_slice[:, 512:1024])
    mv = st.tile([P, 2], F32, tag="mv")
    nc.vector.bn_aggr(mv[:], stats[:])
    rstd = st.tile([P, 1], F32, tag="rstd")
    nc.scalar.activation(rstd[:], mv[:, 1:2], AF.Sqrt, bias=eps_sb[:])
    nc.vector.reciprocal(rstd[:], rstd[:])
    nc.vector.tensor_scalar(
        out=xh[:], in0=h_slice, scalar1=mv[:, 0:1], scalar2=rstd[:],
        op0=OP.subtract, op1=OP.mult,
    )


def _ln_transpose(ctx, nc, tc, hsb, ht, ident, eps_sb):
    """LN all 8 token tiles of hsb and write the transposed result into ht.

    ht: [P, ND, T] fp32r tile ( = x_hat^T ).
    """
    with tc.tile_pool(name="xh", bufs=5) as xhp, \
         tc.tile_pool(name="lnst", bufs=6) as st, \
         tc.tile_pool(name="pst", bufs=2,
                      space=bass.MemorySpace.PSUM) as pst:
        for half in range(2):
            xhs = []
            for q in range(4):
                i = half * 4 + q
                xh = xhp.tile([P, D], F32R, tag="xh")
                _ln_xhat(nc, (st, eps_sb), hsb[:, i, :], xh)
                xhs.append(xh)
            for j in range(ND):
                ps = pst.tile([P, 512], F32R, tag="pst")
                for q in range(4):
                    nc.tensor.transpose(
                        ps[:, q * P:(q + 1) * P],
                        xhs[q][:, j * P:(j + 1) * P],
                        ident[:],
                    )
                nc.vector.tensor_copy(
                    ht[:, j, half * 512:(half + 1) * 512], ps[:])


def _build_nc(n_layers):
    nc = bacc.Bacc("TRN2", target_bir_lowering=False, debug=False,
                   num_devices=8)
    L = n_layers

    h0 = nc.dram_tensor("h0", [T, D], F32, kind="ExternalInput")
    wq = nc.dram_tensor("wq", [L, D, D], F32, kind="ExternalInput")
    wk = nc.dram_tensor("wk", [L, D, D], F32, kind="ExternalInput")
    wv = nc.dram_tensor("wv", [L, D, D], F32, kind="ExternalInput")
    bq = nc.dram_tensor("bq", [L, D], F32, kind="ExternalInput")
    bk = nc.dram_tensor("bk", [L, D], F32, kind="ExternalInput")
    bv = nc.dram_tensor("bv", [L, D], F32, kind="ExternalInput")
    w1 = nc.dram_tensor("w1", [L, HC, ND, P, 512], F32, kind="ExternalInput")
    bg = nc.dram_tensor("bg", [L, HM], F32, kind="ExternalInput")
    w2 = nc.dram_tensor("w2", [L, HM, D], F32, kind="ExternalInput")
    bo = nc.dram_tensor("bo", [L, D], F32, kind="ExternalInput")
    ro = nc.dram_tensor("ro", [D], F32, kind="ExternalInput")
    cst = nc.dram_tensor("cst", [129, 136], F32, kind="ExternalInput")
    pred = nc.dram_tensor("pred", [NI], F32, kind="ExternalOutput")

    with tile.TileContext(nc) as tc:
        import contextlib
        with contextlib.ExitStack() as ctx:
            glob = ctx.enter_context(tc.tile_pool(name="glob", bufs=1))
            hres = ctx.enter_context(tc.tile_pool(name="hres", bufs=1))
            tbig = ctx.enter_context(tc.tile_pool(name="tbig", bufs=1))

            ident = glob.tile([P, P], F32R, tag="ident")
            nc.sync.dma_start(ident[:], cst[0:P, 0:P].bitcast(F32R))
            eps_sb = glob.tile([P, 1], F32, tag="eps")
            nc.vector.memset(eps_sb[:], EPS)
            ones_col = glob.tile([P, 8], F32R, tag="ones_col")
            nc.sync.dma_start(ones_col[:], cst[0:P, 128:136].bitcast(F32R))
            ones_row = glob.tile([1, P], F32R, tag="ones_row")
            nc.sync.dma_start(ones_row[:], cst[128:129, 0:P].bitcast(F32R))

            hsb = hres.tile([P, NT, D], F32, tag="h")
            for i in range(NT):
                nc.sync.dma_start(hsb[:, i, :], h0[i * P:(i + 1) * P, :])

            for l in range(L):
                # ---- per-layer bias tiles ----
                with tc.tile_pool(name="lbias", bufs=1) as lb:
                    bqs = lb.tile([P, ND], F32, tag="bqs")
                    nc.sync.dma_start(
                        bqs[:], bq[l].rearrange("(m p) -> p m", p=P))
                    bks = lb.tile([P, ND], F32, tag="bks")
                    nc.sync.dma_start(
                        bks[:], bk[l].rearrange("(m p) -> p m", p=P))
                    bgs = lb.tile([P, NH], F32, tag="bgs")
                    nc.sync.dma_start(
                        bgs[:], bg[l].rearrange("(m p) -> p m", p=P))
                    bv_bc = lb.tile([P, D], F32, tag="bv_bc")
                    nc.sync.dma_start(bv_bc[:], bv[l][None, :].to_broadcast((P, D)))
                    bo_sb = lb.tile([1, D], F32R, tag="bo_sb")
                    nc.sync.dma_start(bo_sb[:], bo[l][None, :].bitcast(F32R))

                    # ---- LN1 + transpose -> h1t ----
                    h1t = tbig.tile([P, ND, T], F32R, tag="tbig")
                    _ln_transpose(ctx, nc, tc, hsb, h1t, ident, eps_sb)

                    with tc.tile_pool(name="qkv", bufs=2) as qkvp, \
                         tc.tile_pool(name="apool", bufs=1) as apool, \
                         tc.tile_pool(name="wp", bufs=9) as wp, \
                         tc.tile_pool(name="psqkv", bufs=6,
                                      space=bass.MemorySpace.PSUM) as psqkv, \
                         tc.tile_pool(name="psr", bufs=2,
                                      space=bass.MemorySpace.PSUM) as psrp:
                        # ---- Q^T, K^T ----
                        qt = qkvp.tile([P, ND, T], F32R, tag="qkv")
                        kt = qkvp.tile([P, ND, T], F32R, tag="qkv")
                        for wdram, wtile, bias in ((wq, qt, bqs), (wk, kt, bks)):
                            panels = []
                            for k in range(ND):
                                pan = wp.tile([P, D], F32R, tag="wp")
                                nc.sync.dma_start(
                                    pan[:],
                                    wdram[l, k * P:(k + 1) * P, :].bitcast(F32R))
                                panels.append(pan)
                            for n in range(N2):
                                for m in range(ND):
                                    ps = psqkv.tile([P, 512], F32, tag="ps")
                                    for idx in range(ND):
                                        k = (m + idx) % ND
                                        nc.tensor.matmul(
                                            ps[:],
                                            panels[k][:, m * P:(m + 1) * P],
                                            h1t[:, k, n * 512:(n + 1) * 512],
                                            start=(idx == 0), stop=(idx == ND - 1))
                                    nc.vector.tensor_scalar_add(
                                        out=wtile[:, m, n * 512:(n + 1) * 512],
                                        in0=ps[:], scalar1=bias[:, m:m + 1])

                        # ---- scores^T + exp -> a  (a[k_tok, q_tok]) ----
                        a = apool.tile([P, ND, T], F32R, tag="a")
                        for m in range(ND):
                            for n in range(N2):
                                ps = psqkv.tile([P, 512], F32, tag="ps")
                                for d in range(ND):
                                    nc.tensor.matmul(
                                        ps[:],
                                        kt[:, d, m * P:(m + 1) * P],
                                        qt[:, d, n * 512:(n + 1) * 512],
                                        start=(d == 0), stop=(d == ND - 1))
                                nc.scalar.activation(
                                    a[:, m, n * 512:(n + 1) * 512], ps[:],
                                    AF.Exp)

                        # ---- V (natural layout) ----
                        vsb = qkvp.tile([P, ND, T], F32R, tag="qkv")
                        panels = []
                        for k in range(ND):
                            pan = wp.tile([P, D], F32R, tag="wp")
                            nc.sync.dma_start(
                                pan[:],
                                wv[l, k * P:(k + 1) * P, :].bitcast(F32R))
                            panels.append(pan)
                        for n in range(N2):
                            for m in range(ND):
                                ps = psqkv.tile([P, 512], F32, tag="ps")
                                for idx in range(ND):
                                    k = (m + idx) % ND
                                    nc.tensor.matmul(
                                        ps[:],
                                        h1t[:, k, m * P:(m + 1) * P],
                                        panels[k][:, n * 512:(n + 1) * 512],
                                        start=(idx == 0), stop=(idx == ND - 1))
                                nc.vector.tensor_tensor(
                                    out=vsb[:, m, n * 512:(n + 1) * 512],
                                    in0=ps[:],
                                    in1=bv_bc[:, n * 512:(n + 1) * 512],
                                    op=OP.add)

                        # ---- O = softmax-normalized A @ V, H += O ----
                        with tc.tile_pool(name="ost", bufs=4) as ost:
                            for m2 in range(NT):
                                psr = psrp.tile([P, 8], F32, tag="psr")
                                for k in range(ND):
                                    nc.tensor.matmul(
                                        psr[:],
                                        a[:, k, m2 * P:(m2 + 1) * P],
                                        ones_col[:],
                                        start=(k == 0), stop=(k == ND - 1))
                                recip = ost.tile([P, 1], F32, tag="recip")
                                nc.vector.reciprocal(recip[:], psr[:, 0:1])
                                for n in range(N2):
                                    ps = psqkv.tile([P, 512], F32, tag="ps")
                                    for k in range(ND):
                                        nc.tensor.matmul(
                                            ps[:],
                                            a[:, k, m2 * P:(m2 + 1) * P],
                                            vsb[:, k, n * 512:(n + 1) * 512],
                                            start=(k == 0), stop=(k == ND - 1))
                                    nc.scalar.activation(
                                        ps[:], ps[:], AF.Copy,
                                        scale=recip[:])
                                    nc.vector.tensor_add(
                                        hsb[:, m2, n * 512:(n + 1) * 512],
                                        hsb[:, m2, n * 512:(n + 1) * 512],
                                        ps[:])

                    # ---- LN2 + transpose -> h2t ----
                    h2t = tbig.tile([P, ND, T], F32R, tag="tbig")
                    _ln_transpose(ctx, nc, tc, hsb, h2t, ident, eps_sb)

                    # ---- MLP ----
                    with tc.tile_pool(name="gt", bufs=2) as gtp, \
                         tc.tile_pool(name="w1p", bufs=10) as w1p, \
                         tc.tile_pool(name="w2p", bufs=6) as w2p, \
                         tc.tile_pool(name="psg", bufs=3,
                                      space=bass.MemorySpace.PSUM) as psgp, \
                         tc.tile_pool(name="psd", bufs=4,
                                      space=bass.MemorySpace.PSUM) as psdp:
                        for hc in range(HC):
                            blocks = []
                            for k in range(ND):
                                blk = w1p.tile([P, 512], F32R, tag="w1b")
                                nc.sync.dma_start(
                                    blk[:], w1[l, hc, k].bitcast(F32R))
                                blocks.append(blk)
                            gt = gtp.tile([P, 4, T], F32R, tag="gt")
                            for mh in range(4):
                                for n in range(N2):
                                    ps = psgp.tile([P, 512], F32, tag="psg")
                                    for idx in range(ND):
                                        k = (mh * N2 + n + idx) % ND
                                        nc.tensor.matmul(
                                            ps[:],
                                            blocks[k][:, mh * P:(mh + 1) * P],
                                            h2t[:, k, n * 512:(n + 1) * 512],
                                            start=(idx == 0), stop=(idx == ND - 1))
                                    hcol = hc * 4 + mh
                                    nc.scalar.activation(
                                        gt[:, mh, n * 512:(n + 1) * 512],
                                        ps[:], AF.Gelu,
                                        bias=bgs[:, hcol:hcol + 1])
                            w2panels = []
                            for k2 in range(4):
                                pan = w2p.tile([P, D], F32R, tag="w2p")
                                row = hc * 512 + k2 * P
                                nc.sync.dma_start(
                                    pan[:], w2[l, row:row + P, :].bitcast(F32R))
                                w2panels.append(pan)
                            for m2 in range(NT):
                                for n in range(N2):
                                    ps = psdp.tile([P, 512], F32, tag="psd")
                                    for idx in range(4):
                                        k2 = (m2 * N2 + n + idx) % 4
                                        nc.tensor.matmul(
                                            ps[:],
                                            gt[:, k2, m2 * P:(m2 + 1) * P],
                                            w2panels[k2][:, n * 512:(n + 1) * 512],
                                            start=(idx == 0),
                                            stop=(idx == 3 and hc != 0))
                                    if hc == 0:
                                        nc.tensor.matmul(
                                            ps[:], ones_row[:],
                                            bo_sb[:, n * 512:(n + 1) * 512],
                                            start=False, stop=True)
                                    nc.vector.tensor_add(
                                        hsb[:, m2, n * 512:(n + 1) * 512],
                                        hsb[:, m2, n * 512:(n + 1) * 512],
                                        ps[:])

            # ---- head: pred = (H @ ro_w)[:NI] ----
            with tc.tile_pool(name="head", bufs=2) as hp:
                ro_bc = hp.tile([P, D], F32, tag="ro_bc")
                nc.sync.dma_start(ro_bc[:], ro[:][None, :].to_broadcast((P, D)))
                for i in range(NI // P):
                    tmp = hp.tile([P, D], F32, tag="tmp")
                    nc.vector.tensor_mul(tmp[:], hsb[:, i, :], ro_bc[:])
                    pr = hp.tile([P, 1], F32, tag="pr")
                    nc.vector.reduce_sum(pr[:], tmp[:], axis=AX)
                    nc.sync.dma_start(pred[i * P:(i + 1) * P], pr[:])

    nc.compile()
    return nc


def _get_nc(n_layers):
    if n_layers not in _CACHE:
        _CACHE[n_layers] = _build_nc(n_layers)
    return _CACHE[n_layers]


def _prep_inputs(xt, zi, pos_emb, t_emb, Wq, Wk, Wv, ln1_g, ln1_b, ln2_g,
                 ln2_b, mlp_w1, mlp_b1, mlp_w2, mlp_b2, ro_w, ro_b, n_layers):
    L = n_layers
    xt = np.asarray(xt).astype(np.int64)
    zi = np.asarray(zi).astype(np.int64)
    pos_emb = np.asarray(pos_emb, dtype=np.float32)
    t_emb = np.asarray(t_emb, dtype=np.float32)

    opts = np.arange(VOCAB, dtype=np.float32)
    leave = -0.5 * np.square(opts[None, None, :] -
                             zi[:, :, None].astype(np.float32))
    emb_i = np.zeros((B, NI, D), np.float32)
    emb_i[:, :, :VOCAB] = leave
    emb_t = t_emb[xt]
    h0 = np.concatenate([emb_i, emb_t], axis=1) + pos_emb[None]

    g1 = np.asarray(ln1_g, dtype=np.float32)[:L]
    b1 = np.asarray(ln1_b, dtype=np.float32)[:L]
    g2 = np.asarray(ln2_g, dtype=np.float32)[:L]
    b2 = np.asarray(ln2_b, dtype=np.float32)[:L]
    Wq = np.asarray(Wq, dtype=np.float32)[:L]
    Wk = np.asarray(Wk, dtype=np.float32)[:L]
    Wv = np.asarray(Wv, dtype=np.float32)[:L]
    W1 = np.asarray(mlp_w1, dtype=np.float32)[:L]
    W2 = np.asarray(mlp_w2, dtype=np.float32)[:L]
    mb1 = np.asarray(mlp_b1, dtype=np.float32)[:L]
    mb2 = np.asarray(mlp_b2, dtype=np.float32)[:L]

    scale = np.float32(1.0 / np.sqrt(D))
    wq_f = np.ascontiguousarray(g1[:, :, None] * Wq * scale)
    wk_f = np.ascontiguousarray(g1[:, :, None] * Wk)
    wv_f = np.ascontiguousarray(g1[:, :, None] * Wv)
    bq_f = np.einsum('ld,ldo->lo', b1, Wq).astype(np.float32) * scale
    bk_f = np.einsum('ld,ldo->lo', b1, Wk).astype(np.float32)
    bv_f = np.einsum('ld,ldo->lo', b1, Wv).astype(np.float32)
    w1_f = g2[:, :, None] * W1
    bg_f = (np.einsum('ld,ldh->lh', b2, W1) + mb1).astype(np.float32)
    # pre-tile W1 to [L, hc, k, 128, 512] for contiguous DMA blocks
    w1_t = np.ascontiguousarray(
        w1_f.reshape(L, ND, P, HC, 512).transpose(0, 3, 1, 2, 4))

    common = {
        "wq": wq_f, "wk": wk_f, "wv": wv_f,
        "bq": np.ascontiguousarray(bq_f), "bk": np.ascontiguousarray(bk_f),
        "bv": np.ascontiguousarray(bv_f),
        "w1": w1_t, "bg": np.ascontiguousarray(bg_f),
        "w2": np.ascontiguousarray(W2), "bo": np.ascontiguousarray(mb2),
        "ro": np.ascontiguousarray(np.asarray(ro_w, np.float32)[:, 0]),
        "cst": _make_cst(),
    }
    in_maps = []
    for c in range(B):
        m = dict(common)
        m["h0"] = np.ascontiguousarray(h0[c])
        in_maps.append(m)
    return in_maps


def _make_cst():
    cst = np.zeros((129, 136), np.float32)
    cst[0:P, 0:P] = np.eye(P, dtype=np.float32)
    cst[0:P, 128:136] = 1.0
    cst[128, 0:P] = 1.0
    return cst


def _run(inputs, n_layers, **run_kwargs):
    nc = _get_nc(n_layers)
    in_maps = _prep_inputs(n_layers=n_layers, **inputs)
    res = run_bass_kernel_spmd(nc, in_maps, core_ids=list(range(8)),
                               **run_kwargs)
    ro_b = np.asarray(inputs["ro_b"], dtype=np.float32)
    out = np.stack([res.results[c]["pred"] for c in range(B)]) + ro_b[0]
    return out.astype(np.float32), res


def kernel(**inputs):
    out, _ = _run(inputs, n_layers=12)
    return out

